# revision 3
# baseline (speedup 1.0000x reference)
"""ConvGRU Trainium2 kernel (v2).

video [B=2, T=16, C=128, H=64, W=64] f32; 1x1-conv GRU over T.
Sharding: data-parallel over (B x H/16) -> 8 cores, each core owns
P = 16*64 = 1024 pixels for all T; weights replicated.

Per core, channels on partitions, pixels on the free dim, G=2 pixel
groups of PG=512 forming two independent recurrence chains that
interleave on the engines.

v2 changes vs v1 (93.1us):
  - bf16 matmul operands (fp16 moving operands stream at ~1.77
    cycles/col on the PE; bf16 streams at 1 -> MM 378ns -> ~216ns)
  - PSUM layout [z0|z1|r0|r1] (one 4-bank tile): the z-gate pre-acts
    of both groups are contiguous, so one merged [C,1024] sigmoid
    serves both groups (5 ACT ops/step instead of 6 -- ACT is the
    pacing engine at ~4.1us/step of streaming)
  - scalar_tensor_tensor fuses z=1-zbar into the blend:
        u    = zbar*h                      (TT)
        vneg = (zbar-1)*c = -z*c           (STT)
        h'   = u - vneg  = zbar*h + z*c    (TT)
    killing both TENSOR_SCALAR ops per step
  - t=0 shortcut: h0=0 so closers, r-sigmoids and rh are skipped
  - merged per-step state tile [C,1024] -> one output DMA per step,
    issued from the otherwise idle GpSimd queue; x loads keep the
    sync queue; weight DMA split across both queues so x0 lands early
  - PE warmup runs against a memset tile (no weight-DMA dependency)
    so the HAM clock-gate flips during the initial DMAs

Numerics: bf16 matmul inputs/gates/state, fp32 PSUM accum + fp32 bias.
"""

import os
import sys

import numpy as np

B, T, C, H, W = 2, 16, 128, 64, 64
NCORES = 8
HQ = H // 4          # 16 rows of H per core (4 H-slices x 2 batches = 8 cores)
P = HQ * W           # 1024 pixels per core
G = 2                # pixel groups per step (independent recurrence chains)
PG = P // G          # 512 pixels per group

_PROG = None


def _ensure_paths():
    for p in ("/opt/trn_rl_repo",):
        if p not in sys.path and os.path.isdir(p):
            sys.path.append(p)


def _build():
    _ensure_paths()
    import concourse.bacc as bacc
    import concourse.tile as tile
    from concourse import mybir

    f32 = mybir.dt.float32
    bf16 = mybir.dt.bfloat16
    AF = mybir.ActivationFunctionType
    OP = mybir.AluOpType

    nc = bacc.Bacc(
        "TRN2", target_bir_lowering=False, debug=False, num_devices=NCORES
    )
    x_dram = nc.dram_tensor("x_seq", [T, C, P], bf16, kind="ExternalInput")
    w_dram = nc.dram_tensor("wmats", [C, 6 * C], bf16, kind="ExternalInput")
    b_dram = nc.dram_tensor("biases", [C, 4], f32, kind="ExternalInput")
    o_dram = nc.dram_tensor("out_seq", [T, C, P], bf16, kind="ExternalOutput")

    x_ap = x_dram.ap()
    w_ap = w_dram.ap()
    b_ap = b_dram.ap()
    o_ap = o_dram.ap()

    # weight order in wmats: x-side first so its DMA can land first
    WZX, WRX, WHX, WZH, WRH, WHH = range(6)

    with tile.TileContext(nc) as tc:
        with (
            tc.tile_pool(name="consts", bufs=1) as consts,
            tc.tile_pool(name="xin", bufs=4) as xpool,
            tc.tile_pool(name="state", bufs=3) as spool,
            tc.tile_pool(name="work", bufs=3) as wk,
            tc.tile_pool(name="ps", bufs=1, space="PSUM") as ps,
        ):
            wt = consts.tile([C, 6 * C], bf16)
            bt = consts.tile([C, 4], f32)
            # x-side weights + first x tiles on the sync HW queue;
            # h-side weights + biases on the gpsimd queue (parallel)
            nc.sync.dma_start(wt[:, : 3 * C], w_ap[:, : 3 * C])
            nc.gpsimd.dma_start(bt[:], b_ap[:])
            nc.gpsimd.dma_start(wt[:, 3 * C :], w_ap[:, 3 * C :])

            def wslice(i):
                return wt[:, i * C : (i + 1) * C]

            def load_x(t):
                xt = xpool.tile([C, P], bf16, tag="x")
                nc.sync.dma_start(xt[:], x_ap[t])
                return xt

            x_tiles = {0: load_x(0), 1: load_x(1), 2: load_x(2)}

            # PSUM: [z0|z1|r0|r1] in one 4-bank tile + double-buffered c
            zr = ps.tile([C, 4 * PG], f32, tag="zr")

            def zsl(g):
                return zr[:, g * PG : (g + 1) * PG]

            def rsl(g):
                return zr[:, (2 + g) * PG : (3 + g) * PG]

            # -- warmup: ramp the PE clock gate with matmuls that only
            #    depend on a memset tile, while the input DMAs fly --
            warm16 = wk.tile([C, PG], bf16, tag="warm")
            nc.vector.memset(warm16[:], 0.0)
            cwarm = [None, None]
            for g in range(G):
                cwarm[g] = ps.tile(
                    [C, PG], f32, tag=f"c_{g}", bufs=2, name=f"cwarm_{g}"
                )
            for i in range(6):
                nc.tensor.matmul(
                    cwarm[i % 2][:], warm16[:, :C], warm16[:],
                    start=True, stop=True,
                )
            # preload the ACT sigmoid/tanh table early
            wtmp = wk.tile([C, PG], bf16, tag="scratch")
            nc.scalar.activation(
                wtmp[:], cwarm[0][:], AF.Sigmoid, bias=bt[:, 0:1]
            )

            h_prev = None  # merged [C, P] state tile of step t-1

            # ---- t = 0: h0 == 0, so no closers / r-gate / rh ----
            x0 = x_tiles[0]
            for g in range(G):
                nc.tensor.matmul(
                    zsl(g), wslice(WZX), x0[:, g * PG : (g + 1) * PG],
                    start=True, stop=True,
                )
            c0 = [None, None]
            for g in range(G):
                cp = ps.tile([C, PG], f32, tag=f"c_{g}", bufs=2)
                nc.tensor.matmul(
                    cp[:], wslice(WHX), x0[:, g * PG : (g + 1) * PG],
                    start=True, stop=True,
                )
                c0[g] = cp
            z16m = wk.tile([C, 2 * PG], bf16, tag="zb")
            nc.scalar.activation(
                z16m[:], zr[:, : 2 * PG], AF.Sigmoid, bias=bt[:, 0:1]
            )
            h_new = spool.tile([C, P], bf16, tag="h")
            for g in range(G):
                ct = wk.tile([C, PG], bf16, tag=f"c16_{g}")
                nc.scalar.activation(ct[:], c0[g][:], AF.Tanh, bias=bt[:, 2:3])
                nc.vector.tensor_mul(
                    h_new[:, g * PG : (g + 1) * PG],
                    z16m[:, g * PG : (g + 1) * PG], ct[:],
                )
            nc.gpsimd.dma_start(o_ap[0], h_new[:])
            h_prev = h_new

            # openers for t=1
            x1 = x_tiles[1]
            for g in range(G):
                xs = x1[:, g * PG : (g + 1) * PG]
                nc.tensor.matmul(zsl(g), wslice(WZX), xs, start=True, stop=False)
                nc.tensor.matmul(rsl(g), wslice(WRX), xs, start=True, stop=False)
            c_t = [None, None]
            for g in range(G):
                cp = ps.tile([C, PG], f32, tag=f"c_{g}", bufs=2)
                nc.tensor.matmul(
                    cp[:], wslice(WHX), x1[:, g * PG : (g + 1) * PG],
                    start=True, stop=False,
                )
                c_t[g] = cp

            # ---- steady steps t = 1..T-1 ----
            for t in range(1, T):
                go = [0, 1] if t % 2 == 1 else [1, 0]
                if t + 2 < T and (t + 2) not in x_tiles:
                    pass
                x_next = x_tiles.get(t + 1)
                # prefetch x_{t+2}
                if t + 2 < T:
                    x_tiles[t + 2] = load_x(t + 2)

                def hsl(g):
                    return h_prev[:, g * PG : (g + 1) * PG]

                # -- PE: close r (chain head), then z --
                for g in go:
                    nc.tensor.matmul(
                        rsl(g), wslice(WRH), hsl(g), start=False, stop=True
                    )
                for g in go:
                    nc.tensor.matmul(
                        zsl(g), wslice(WZH), hsl(g), start=False, stop=True
                    )

                # -- ACT: r sigmoids first (they gate rh -> c matmul) --
                r16 = [None, None]
                for g in go:
                    rt = wk.tile([C, PG], bf16, tag=f"r_{g}")
                    nc.scalar.activation(
                        rt[:], rsl(g), AF.Sigmoid, bias=bt[:, 1:2]
                    )
                    r16[g] = rt

                rh16 = [None, None]
                for g in go:
                    rh = wk.tile([C, PG], bf16, tag=f"rh_{g}")
                    nc.vector.tensor_mul(rh[:], r16[g][:], hsl(g))
                    rh16[g] = rh

                for g in go:
                    nc.tensor.matmul(
                        c_t[g][:], wslice(WHH), rh16[g][:],
                        start=False, stop=True,
                    )

                # next step's c openers (double-buffered, off-chain)
                c_next = [None, None]
                if x_next is not None and t + 1 < T:
                    for g in go:
                        cp = ps.tile([C, PG], f32, tag=f"c_{g}", bufs=2)
                        nc.tensor.matmul(
                            cp[:], wslice(WHX),
                            x_next[:, g * PG : (g + 1) * PG],
                            start=True, stop=False,
                        )
                        c_next[g] = cp

                # -- ACT: merged zbar over both groups, then tanh --
                zb16 = wk.tile([C, 2 * PG], bf16, tag="zb")
                nc.scalar.activation(
                    zb16[:], zr[:, : 2 * PG], AF.Sigmoid,
                    bias=bt[:, 3:4], scale=-1.0,
                )

                # u = zbar*h overlaps the tanh
                u16 = [None, None]
                for g in go:
                    ut = wk.tile([C, PG], bf16, tag=f"u_{g}")
                    nc.vector.tensor_mul(
                        ut[:], zb16[:, g * PG : (g + 1) * PG], hsl(g)
                    )
                    u16[g] = ut

                c16 = [None, None]
                for g in go:
                    ct = wk.tile([C, PG], bf16, tag=f"c16_{g}")
                    nc.scalar.activation(
                        ct[:], c_t[g][:], AF.Tanh, bias=bt[:, 2:3]
                    )
                    c16[g] = ct

                # next step's z|r openers (wait on this step's sigmoids)
                if x_next is not None and t + 1 < T:
                    for g in go:
                        xs = x_next[:, g * PG : (g + 1) * PG]
                        nc.tensor.matmul(
                            rsl(g), wslice(WRX), xs, start=True, stop=False
                        )
                        nc.tensor.matmul(
                            zsl(g), wslice(WZX), xs, start=True, stop=False
                        )

                # -- DVE tail: vneg = (zbar-1)*c = -z*c ; h' = u - vneg --
                h_new = spool.tile([C, P], bf16, tag="h")
                for g in go:
                    vn = wk.tile([C, PG], bf16, tag=f"v_{g}")
                    nc.vector.scalar_tensor_tensor(
                        vn[:], zb16[:, g * PG : (g + 1) * PG], 1.0,
                        c16[g][:], OP.subtract, OP.mult,
                    )
                    nc.vector.tensor_sub(
                        h_new[:, g * PG : (g + 1) * PG], u16[g][:], vn[:]
                    )
                nc.gpsimd.dma_start(o_ap[t], h_new[:])

                h_prev = h_new
                x_tiles.pop(t - 1, None)
                c_t = c_next

    nc.compile()
    return nc


def _get_prog():
    global _PROG
    if _PROG is None:
        _PROG = _build()
    return _PROG


def _make_in_maps(video, Wz, bz, Wr, br, Wh, bh):
    import ml_dtypes

    bf = ml_dtypes.bfloat16
    w6 = np.concatenate(
        [
            Wz[:, :C].T, Wr[:, :C].T, Wh[:, :C].T,
            Wz[:, C:].T, Wr[:, C:].T, Wh[:, C:].T,
        ],
        axis=1,
    ).astype(bf)
    b3 = np.stack([bz, br, bh, -bz], axis=1).astype(np.float32)
    in_maps = []
    for core in range(NCORES):
        b_, q = divmod(core, 4)
        xs = np.ascontiguousarray(
            video[b_, :, :, q * HQ : (q + 1) * HQ, :]
        ).reshape(T, C, P).astype(bf)
        in_maps.append({"x_seq": xs, "wmats": w6, "biases": b3})
    return in_maps


def kernel(video, Wz, bz, Wr, br, Wh, bh):
    _ensure_paths()
    from concourse.bass_utils import run_bass_kernel_spmd

    video = np.asarray(video, dtype=np.float32)
    nc = _get_prog()
    in_maps = _make_in_maps(video, Wz, bz, Wr, br, Wh, bh)
    res = run_bass_kernel_spmd(nc, in_maps, list(range(NCORES)))

    out = np.empty((B, T, C, H, W), np.float32)
    for core in range(NCORES):
        b_, q = divmod(core, 4)
        out[b_, :, :, q * HQ : (q + 1) * HQ, :] = np.asarray(
            res.results[core]["out_seq"]
        ).astype(np.float32).reshape(T, C, HQ, W)
    return out


# revision 4
# speedup vs baseline: 1.2070x; 1.2070x over previous
"""ConvGRU Trainium2 kernel (v3).

video [B=2, T=16, C=128, H=64, W=64] f32; 1x1-conv GRU over T.
Sharding: data-parallel over (B x H/16) -> 8 cores, each core owns
P = 16*64 = 1024 pixels for all T; weights replicated.

Per core, channels on partitions, pixels on the free dim, G=2 pixel
groups of PG=512 forming two independent recurrence chains that
interleave on the engines.

Per group, per timestep (zbar = 1-z, computed as sigmoid(-pre)):
    r  = sigmoid(pre_r + br)               (ACT, on the chain)
    rh = r * h                             (DVE, on the chain)
    c  = tanh(Whx@x + Whh@rh + bh)         (PE + ACT, on the chain)
    u  = zbar * h ; z = 1 - zbar           (DVE, overlaps the tanh)
    h' = u + z * c                         (DVE tail)

Design notes (evidence from NTFF traces):
  - fp16 everywhere: bf16 ops measure uniformly slower on this stack
    (ACTIVATE 687->823ns, TT 423->508ns, MM latency +75ns), and fp16
    matmuls already pipeline at the 216ns/MM N=512 roofline.
  - The Scalar (ACT) queue is the pacing engine (~4us/step of
    streaming).  PSUM layout [z0|z1|r0|r1] (one 4-bank tile) makes
    both groups' z-gate pre-acts contiguous, so one merged [C,1024]
    sigmoid serves both groups: 5 ACT ops/step instead of 6.  The
    z = 1-zbar TENSOR_SCALAR also merges to one [C,1024] op (4x mode).
  - r-sigmoids come first on ACT (they gate the rh -> Whh matmul);
    tanh stays per-group so each group's blend starts early; the
    merged zbar slots between them (z path has slack until u/v).
  - t=0 shortcut: h0 == 0, so closers, r-sigmoid and rh are skipped
    and h1 = sigmoid(pre_z + bz) * tanh(pre_c + bh).
  - Output DMAs ride the otherwise-idle GpSimd queue; the sync HW
    queue keeps x prefetches only.  The weight DMA is split across
    both queues (x-side first) so x0 lands early.
  - PE warmup matmuls run against a memset tile (no weight-DMA
    dependency) flipping the HAM clock gate during the initial DMAs.

Numerics: fp16 matmul inputs/gates/state, fp32 PSUM accum + fp32 bias.
"""

import os
import sys

import numpy as np

B, T, C, H, W = 2, 16, 128, 64, 64
NCORES = 8
HQ = H // 4          # 16 rows of H per core (4 H-slices x 2 batches = 8 cores)
P = HQ * W           # 1024 pixels per core
G = 2                # pixel groups per step (independent recurrence chains)
PG = P // G          # 512 pixels per group

_PROG = None


def _ensure_paths():
    for p in ("/opt/trn_rl_repo",):
        if p not in sys.path and os.path.isdir(p):
            sys.path.append(p)


def _build():
    _ensure_paths()
    import concourse.bacc as bacc
    import concourse.tile as tile
    from concourse import mybir

    f32 = mybir.dt.float32
    f16 = mybir.dt.float16
    AF = mybir.ActivationFunctionType

    nc = bacc.Bacc(
        "TRN2", target_bir_lowering=False, debug=False, num_devices=NCORES
    )
    x_dram = nc.dram_tensor("x_seq", [T, C, P], f16, kind="ExternalInput")
    w_dram = nc.dram_tensor("wmats", [C, 6 * C], f16, kind="ExternalInput")
    b_dram = nc.dram_tensor("biases", [C, 4], f32, kind="ExternalInput")
    o_dram = nc.dram_tensor("out_seq", [T, C, P], f16, kind="ExternalOutput")

    x_ap = x_dram.ap()
    w_ap = w_dram.ap()
    b_ap = b_dram.ap()
    o_ap = o_dram.ap()

    # weight order in wmats: x-side first so its DMA can land first
    WZX, WRX, WHX, WZH, WRH, WHH = range(6)

    with tile.TileContext(nc) as tc:
        with (
            tc.tile_pool(name="consts", bufs=1) as consts,
            tc.tile_pool(name="xin", bufs=4) as xpool,
            tc.tile_pool(name="state", bufs=3) as spool,
            tc.tile_pool(name="work", bufs=3) as wk,
            tc.tile_pool(name="ps", bufs=1, space="PSUM") as ps,
        ):
            wt = consts.tile([C, 6 * C], f16)
            bt = consts.tile([C, 4], f32)
            # x-side weights + x tiles on the sync HW queue;
            # h-side weights + biases on the gpsimd queue (parallel)
            nc.sync.dma_start(wt[:, : 3 * C], w_ap[:, : 3 * C])
            nc.gpsimd.dma_start(bt[:], b_ap[:])
            nc.gpsimd.dma_start(wt[:, 3 * C :], w_ap[:, 3 * C :])

            def wslice(i):
                return wt[:, i * C : (i + 1) * C]

            def load_x(t):
                xt = xpool.tile([C, P], f16, tag="x")
                nc.sync.dma_start(xt[:], x_ap[t])
                return xt

            x_tiles = {0: load_x(0), 1: load_x(1), 2: load_x(2)}

            # PSUM: [z0|z1|r0|r1] in one 4-bank tile + double-buffered c
            zr = ps.tile([C, 4 * PG], f32, tag="zr")

            def zsl(g):
                return zr[:, g * PG : (g + 1) * PG]

            def rsl(g):
                return zr[:, (2 + g) * PG : (3 + g) * PG]

            # -- warmup: ramp the PE clock gate with matmuls that only
            #    depend on a memset tile, while the input DMAs fly --
            warm16 = wk.tile([C, PG], f16, tag="warm")
            nc.vector.memset(warm16[:], 0.0)
            cwarm = [None, None]
            for g in range(G):
                cwarm[g] = ps.tile(
                    [C, PG], f32, tag=f"c_{g}", bufs=2, name=f"cwarm_{g}"
                )
            for i in range(6):
                nc.tensor.matmul(
                    cwarm[i % 2][:], warm16[:, :C], warm16[:],
                    start=True, stop=True,
                )
            # preload the ACT sigmoid/tanh table early
            wtmp = wk.tile([C, PG], f16, tag="scratch")
            nc.scalar.activation(
                wtmp[:], cwarm[0][:], AF.Sigmoid, bias=bt[:, 0:1]
            )

            # ---- t = 0: h0 == 0, so no closers / r-gate / rh ----
            x0 = x_tiles[0]
            for g in range(G):
                nc.tensor.matmul(
                    zsl(g), wslice(WZX), x0[:, g * PG : (g + 1) * PG],
                    start=True, stop=True,
                )
            c0 = [None, None]
            for g in range(G):
                cp = ps.tile([C, PG], f32, tag=f"c_{g}", bufs=2)
                nc.tensor.matmul(
                    cp[:], wslice(WHX), x0[:, g * PG : (g + 1) * PG],
                    start=True, stop=True,
                )
                c0[g] = cp
            z16m = wk.tile([C, 2 * PG], f16, tag="zb")
            nc.scalar.activation(
                z16m[:], zr[:, : 2 * PG], AF.Sigmoid, bias=bt[:, 0:1]
            )
            h16 = [None, None]
            for g in range(G):
                ct = wk.tile([C, PG], f16, tag=f"c16_{g}")
                nc.scalar.activation(ct[:], c0[g][:], AF.Tanh, bias=bt[:, 2:3])
                ht = spool.tile([C, PG], f16, tag=f"h16_{g}")
                nc.vector.tensor_mul(
                    ht[:], z16m[:, g * PG : (g + 1) * PG], ct[:]
                )
                h16[g] = ht
                nc.gpsimd.dma_start(
                    o_ap[0, :, g * PG : (g + 1) * PG], ht[:]
                )

            # openers for t=1
            x1 = x_tiles[1]
            for g in range(G):
                xs = x1[:, g * PG : (g + 1) * PG]
                nc.tensor.matmul(rsl(g), wslice(WRX), xs, start=True, stop=False)
                nc.tensor.matmul(zsl(g), wslice(WZX), xs, start=True, stop=False)
            c_t = [None, None]
            for g in range(G):
                cp = ps.tile([C, PG], f32, tag=f"c_{g}", bufs=2)
                nc.tensor.matmul(
                    cp[:], wslice(WHX), x1[:, g * PG : (g + 1) * PG],
                    start=True, stop=False,
                )
                c_t[g] = cp

            # ---- steady steps t = 1..T-1 ----
            for t in range(1, T):
                go = [0, 1] if t % 2 == 1 else [1, 0]
                x_next = x_tiles.get(t + 1)
                if t + 2 < T:
                    x_tiles[t + 2] = load_x(t + 2)

                # -- PE: close r (chain head), then z --
                for g in go:
                    nc.tensor.matmul(
                        rsl(g), wslice(WRH), h16[g][:], start=False, stop=True
                    )
                for g in go:
                    nc.tensor.matmul(
                        zsl(g), wslice(WZH), h16[g][:], start=False, stop=True
                    )

                # -- ACT: r sigmoids first (they gate rh -> c matmul) --
                r16 = [None, None]
                for g in go:
                    rt = wk.tile([C, PG], f16, tag=f"r_{g}")
                    nc.scalar.activation(
                        rt[:], rsl(g), AF.Sigmoid, bias=bt[:, 1:2]
                    )
                    r16[g] = rt

                rh16 = [None, None]
                for g in go:
                    rh = wk.tile([C, PG], f16, tag=f"rh_{g}")
                    nc.vector.tensor_mul(rh[:], r16[g][:], h16[g][:])
                    rh16[g] = rh

                for g in go:
                    nc.tensor.matmul(
                        c_t[g][:], wslice(WHH), rh16[g][:],
                        start=False, stop=True,
                    )

                # next step's c openers (double-buffered, off-chain)
                c_next = [None, None]
                if x_next is not None:
                    for g in go:
                        cp = ps.tile([C, PG], f32, tag=f"c_{g}", bufs=2)
                        nc.tensor.matmul(
                            cp[:], wslice(WHX),
                            x_next[:, g * PG : (g + 1) * PG],
                            start=True, stop=False,
                        )
                        c_next[g] = cp

                # -- ACT: merged zbar over both groups --
                zb16 = wk.tile([C, 2 * PG], f16, tag="zb")
                nc.scalar.activation(
                    zb16[:], zr[:, : 2 * PG], AF.Sigmoid,
                    bias=bt[:, 3:4], scale=-1.0,
                )

                # z = 1 - zbar, merged over both groups (4x-mode TS);
                # u = zbar*h overlaps the tanh
                z16 = wk.tile([C, 2 * PG], f16, tag="z")
                nc.vector.tensor_scalar(
                    z16[:], zb16[:], -1.0, 1.0,
                    mybir.AluOpType.mult, mybir.AluOpType.add,
                )
                u16 = [None, None]
                for g in go:
                    ut = wk.tile([C, PG], f16, tag=f"u_{g}")
                    nc.vector.tensor_mul(
                        ut[:], zb16[:, g * PG : (g + 1) * PG], h16[g][:]
                    )
                    u16[g] = ut

                c16 = [None, None]
                for g in go:
                    ct = wk.tile([C, PG], f16, tag=f"c16_{g}")
                    nc.scalar.activation(
                        ct[:], c_t[g][:], AF.Tanh, bias=bt[:, 2:3]
                    )
                    c16[g] = ct

                # next step's z|r openers (wait on this step's sigmoids)
                if x_next is not None:
                    for g in go:
                        xs = x_next[:, g * PG : (g + 1) * PG]
                        nc.tensor.matmul(
                            rsl(g), wslice(WRX), xs, start=True, stop=False
                        )
                        nc.tensor.matmul(
                            zsl(g), wslice(WZX), xs, start=True, stop=False
                        )

                # -- DVE tail: v = z*c ; h' = u + v --
                for g in go:
                    v16 = wk.tile([C, PG], f16, tag=f"v_{g}")
                    nc.vector.tensor_mul(
                        v16[:], z16[:, g * PG : (g + 1) * PG], c16[g][:]
                    )
                    n16 = spool.tile([C, PG], f16, tag=f"h16_{g}")
                    nc.vector.tensor_add(n16[:], u16[g][:], v16[:])
                    h16[g] = n16
                    nc.gpsimd.dma_start(
                        o_ap[t, :, g * PG : (g + 1) * PG], n16[:]
                    )

                x_tiles.pop(t - 1, None)
                c_t = c_next

    nc.compile()
    return nc


def _get_prog():
    global _PROG
    if _PROG is None:
        _PROG = _build()
    return _PROG


def _make_in_maps(video, Wz, bz, Wr, br, Wh, bh):
    w6 = np.concatenate(
        [
            Wz[:, :C].T, Wr[:, :C].T, Wh[:, :C].T,
            Wz[:, C:].T, Wr[:, C:].T, Wh[:, C:].T,
        ],
        axis=1,
    ).astype(np.float16)
    b3 = np.stack([bz, br, bh, -bz], axis=1).astype(np.float32)
    in_maps = []
    for core in range(NCORES):
        b_, q = divmod(core, 4)
        xs = np.ascontiguousarray(
            video[b_, :, :, q * HQ : (q + 1) * HQ, :]
        ).reshape(T, C, P).astype(np.float16)
        in_maps.append({"x_seq": xs, "wmats": w6, "biases": b3})
    return in_maps


def kernel(video, Wz, bz, Wr, br, Wh, bh):
    _ensure_paths()
    from concourse.bass_utils import run_bass_kernel_spmd

    video = np.asarray(video, dtype=np.float32)
    nc = _get_prog()
    in_maps = _make_in_maps(video, Wz, bz, Wr, br, Wh, bh)
    res = run_bass_kernel_spmd(nc, in_maps, list(range(NCORES)))

    out = np.empty((B, T, C, H, W), np.float32)
    for core in range(NCORES):
        b_, q = divmod(core, 4)
        out[b_, :, :, q * HQ : (q + 1) * HQ, :] = np.asarray(
            res.results[core]["out_seq"]
        ).astype(np.float32).reshape(T, C, HQ, W)
    return out


# revision 5
# speedup vs baseline: 1.6349x; 1.3545x over previous
"""ConvGRU Trainium2 kernel (v4).

video [B=2, T=16, C=128, H=64, W=64] f32; 1x1-conv GRU over T.
Sharding: data-parallel over (B x H/16) -> 8 cores, each core owns
P = 16*64 = 1024 pixels for all T; weights replicated.

Per core, per timestep (pixels on the free dim, channels on partitions):
    zr_pre = [Wzx@x + Wzh@h | Wrx@x + Wrh@h]      (PE, fp16 in / fp32 psum)
    z = sigmoid(zr_pre[:P] + bz); r = sigmoid(zr_pre[P:] + br)   (ACT)
    rh = r * h                                     (DVE)
    c = tanh(Whx@x + Whh@rh + bh)                  (PE + ACT)
    h = h + z * (c - h)                            (DVE, fp16 state)

The recurrence is latency-bound: each pixel group's step is a serial
cross-engine chain (h -> Wrh matmul -> sigmoid -> r*h -> Whh matmul ->
tanh -> blend -> h').  G=2 pixel groups form two independent chains
that interleave on the engines; all per-step ops stay PER-GROUP (a
merged-op variant that coupled the chains measured 33% slower).

Changes vs the 93.1us baseline (v1):
  - t=0 shortcut: h0 == 0, so closers, r-sigmoid and rh are skipped
    and h1 = sigmoid(pre_z + bz) * tanh(pre_c + bh)
  - output DMAs ride the otherwise-idle GpSimd queue so x prefetches
    never queue behind them on the sync HW queue
  - x prefetch issued two steps ahead (DMA landing latency is about
    one whole step period)
  - weight DMA split across the sync + gpsimd queues (x-side first)
    so x0 lands earlier; PE warmup matmuls run against a memset tile
    (no weight dependency) flipping the HAM clock gate during the DMAs
  - work pool bufs=3 so WAR buffer-recycle waits are stale and cheap

Numerics: fp16 matmul inputs/gates/state, fp32 PSUM accum + fp32 bias.
"""

import os
import sys

import numpy as np

B, T, C, H, W = 2, 16, 128, 64, 64
NCORES = 8
HQ = H // 4          # 16 rows of H per core (4 H-slices x 2 batches = 8 cores)
P = HQ * W           # 1024 pixels per core
G = 2                # pixel groups per step (independent recurrence chains)
PG = P // G          # 512 pixels per group

_PROG = None


def _ensure_paths():
    for p in ("/opt/trn_rl_repo",):
        if p not in sys.path and os.path.isdir(p):
            sys.path.append(p)


def _build():
    _ensure_paths()
    import concourse.bacc as bacc
    import concourse.tile as tile
    from concourse import mybir

    f32 = mybir.dt.float32
    f16 = mybir.dt.float16
    AF = mybir.ActivationFunctionType

    nc = bacc.Bacc(
        "TRN2", target_bir_lowering=False, debug=False, num_devices=NCORES
    )
    x_dram = nc.dram_tensor("x_seq", [T, C, P], f16, kind="ExternalInput")
    w_dram = nc.dram_tensor("wmats", [C, 6 * C], f16, kind="ExternalInput")
    b_dram = nc.dram_tensor("biases", [C, 4], f32, kind="ExternalInput")
    o_dram = nc.dram_tensor("out_seq", [T, C, P], f16, kind="ExternalOutput")

    x_ap = x_dram.ap()
    w_ap = w_dram.ap()
    b_ap = b_dram.ap()
    o_ap = o_dram.ap()

    # weight order in wmats: x-side first so its DMA can land first
    WZX, WRX, WHX, WZH, WRH, WHH = range(6)

    with tile.TileContext(nc) as tc:
        with (
            tc.tile_pool(name="consts", bufs=1) as consts,
            tc.tile_pool(name="xin", bufs=4) as xpool,
            tc.tile_pool(name="state", bufs=2) as spool,
            tc.tile_pool(name="work", bufs=3) as wk,
            tc.tile_pool(name="ps", bufs=1, space="PSUM") as ps,
        ):
            wt = consts.tile([C, 6 * C], f16)
            bt = consts.tile([C, 4], f32)
            nc.sync.dma_start(wt[:, : 3 * C], w_ap[:, : 3 * C])
            nc.gpsimd.dma_start(bt[:], b_ap[:])
            nc.gpsimd.dma_start(wt[:, 3 * C :], w_ap[:, 3 * C :])

            def wslice(i):
                return wt[:, i * C : (i + 1) * C]

            def load_x(t):
                xt = xpool.tile([C, P], f16, tag="x")
                nc.sync.dma_start(xt[:], x_ap[t])
                return xt

            x_tiles = {0: load_x(0), 1: load_x(1), 2: load_x(2)}

            # -- warmup: ramp the PE clock gate with matmuls that only
            #    depend on a memset tile, while the input DMAs fly --
            warm16 = wk.tile([C, PG], f16, tag="warm")
            nc.vector.memset(warm16[:], 0.0)
            cwarm = [None, None]
            for g in range(G):
                cwarm[g] = ps.tile(
                    [C, PG], f32, tag=f"c_{g}", bufs=2, name=f"cwarm_{g}"
                )
            for i in range(6):
                nc.tensor.matmul(
                    cwarm[i % 2][:], warm16[:, :C], warm16[:],
                    start=True, stop=True,
                )
            # preload the ACT sigmoid/tanh table early
            wtmp = wk.tile([C, PG], f16, tag="scratch")
            nc.scalar.activation(
                wtmp[:], cwarm[0][:], AF.Sigmoid, bias=bt[:, 0:1]
            )

            # ---- t = 0: h0 == 0, so no closers / r-gate / rh ----
            x0 = x_tiles[0]
            zr0 = [None, None]
            for g in range(G):
                zrt = ps.tile([C, 2 * PG], f32, tag=f"zr_{g}", name=f"zr0_{g}")
                nc.tensor.matmul(
                    zrt[:, :PG], wslice(WZX), x0[:, g * PG : (g + 1) * PG],
                    start=True, stop=True,
                )
                zr0[g] = zrt
            c0 = [None, None]
            for g in range(G):
                cp = ps.tile([C, PG], f32, tag=f"c_{g}", bufs=2)
                nc.tensor.matmul(
                    cp[:], wslice(WHX), x0[:, g * PG : (g + 1) * PG],
                    start=True, stop=True,
                )
                c0[g] = cp
            h16 = [None, None]
            for g in range(G):
                zt = wk.tile([C, PG], f16, tag=f"zb_{g}")
                nc.scalar.activation(
                    zt[:], zr0[g][:, :PG], AF.Sigmoid, bias=bt[:, 0:1]
                )
                ct = wk.tile([C, PG], f16, tag=f"c16_{g}")
                nc.scalar.activation(ct[:], c0[g][:], AF.Tanh, bias=bt[:, 2:3])
                ht = spool.tile([C, PG], f16, tag=f"h16_{g}")
                nc.vector.tensor_mul(ht[:], zt[:], ct[:])
                h16[g] = ht
                nc.gpsimd.dma_start(
                    o_ap[0, :, g * PG : (g + 1) * PG], ht[:]
                )

            def open_zr(xt, gorder):
                """Open z|r accumulations with the x-side contributions."""
                zr_t = [None] * G
                for g in gorder:
                    xs = xt[:, g * PG : (g + 1) * PG]
                    zrt = ps.tile([C, 2 * PG], f32, tag=f"zr_{g}", bufs=1,
                                  name=f"zr_t{g}")
                    nc.tensor.matmul(
                        zrt[:, PG:], wslice(WRX), xs, start=True, stop=False
                    )
                    nc.tensor.matmul(
                        zrt[:, :PG], wslice(WZX), xs, start=True, stop=False
                    )
                    zr_t[g] = zrt
                return zr_t

            def open_c(xt, gorder):
                cp_t = [None] * G
                for g in gorder:
                    xs = xt[:, g * PG : (g + 1) * PG]
                    cp = ps.tile([C, PG], f32, tag=f"c_{g}", bufs=2,
                                 name=f"c_t{g}")
                    nc.tensor.matmul(
                        cp[:], wslice(WHX), xs, start=True, stop=False
                    )
                    cp_t[g] = cp
                return cp_t

            first = list(range(G))
            x1 = x_tiles[1]
            zr_t = open_zr(x1, first)
            cp_t = open_c(x1, first)

            for t in range(1, T):
                go = first if t % 2 == 1 else first[::-1]
                x_next = x_tiles.get(t + 1)
                if t + 2 < T:
                    x_tiles[t + 2] = load_x(t + 2)

                # -- PE: close the r then z accumulations (chain head) --
                for g in go:
                    nc.tensor.matmul(
                        zr_t[g][:, PG:], wslice(WRH), h16[g][:],
                        start=False, stop=True,
                    )
                for g in go:
                    nc.tensor.matmul(
                        zr_t[g][:, :PG], wslice(WZH), h16[g][:],
                        start=False, stop=True,
                    )

                # -- ACT: r sigmoids first (they gate rh -> c matmul) --
                r16 = [None] * G
                for g in go:
                    rt = wk.tile([C, PG], f16, tag=f"r_{g}")
                    nc.scalar.activation(
                        rt[:], zr_t[g][:, PG:], AF.Sigmoid, bias=bt[:, 1:2]
                    )
                    r16[g] = rt

                rh16 = [None] * G
                for g in go:
                    rh = wk.tile([C, PG], f16, tag=f"rh_{g}")
                    nc.vector.tensor_mul(rh[:], r16[g][:], h16[g][:])
                    rh16[g] = rh

                for g in go:
                    nc.tensor.matmul(
                        cp_t[g][:], wslice(WHH), rh16[g][:],
                        start=False, stop=True,
                    )

                # next step's c openers can run any time (double-buffered)
                cp_next = open_c(x_next, go) if x_next is not None else None

                # -- ACT: zbar/tanh interleaved; zbar = 1-z = sigmoid(-pre)
                #    feeds the blend h' = zbar*h + (1-zbar)*c, whose only
                #    post-tanh serial ops are v = z*c and h' = u + v --
                zb16, c16 = [None] * G, [None] * G
                for g in go:
                    zbt = wk.tile([C, PG], f16, tag=f"zb_{g}")
                    nc.scalar.activation(
                        zbt[:], zr_t[g][:, :PG], AF.Sigmoid,
                        bias=bt[:, 3:4], scale=-1.0,
                    )
                    zb16[g] = zbt
                    ct = wk.tile([C, PG], f16, tag=f"c16_{g}")
                    nc.scalar.activation(
                        ct[:], cp_t[g][:], AF.Tanh, bias=bt[:, 2:3]
                    )
                    c16[g] = ct

                # next step's z|r openers (wait on this step's sigmoids)
                zr_next = open_zr(x_next, go) if x_next is not None else None

                # -- DVE mid-chain: u = zbar*h and z = 1-zbar overlap the
                #    tanh; only v and the final add trail it --
                u16, z16 = [None] * G, [None] * G
                for g in go:
                    ut = wk.tile([C, PG], f16, tag=f"u_{g}")
                    nc.vector.tensor_mul(ut[:], zb16[g][:], h16[g][:])
                    u16[g] = ut
                    zt = wk.tile([C, PG], f16, tag=f"z_{g}")
                    nc.vector.tensor_scalar(
                        zt[:], zb16[g][:], -1.0, 1.0,
                        mybir.AluOpType.mult, mybir.AluOpType.add,
                    )
                    z16[g] = zt

                for g in go:
                    v16 = wk.tile([C, PG], f16, tag=f"v_{g}")
                    nc.vector.tensor_mul(v16[:], z16[g][:], c16[g][:])
                    n16 = spool.tile([C, PG], f16, tag=f"h16_{g}")
                    nc.vector.tensor_add(n16[:], u16[g][:], v16[:])
                    h16[g] = n16
                    nc.gpsimd.dma_start(
                        o_ap[t, :, g * PG : (g + 1) * PG], n16[:]
                    )

                x_tiles.pop(t - 1, None)
                if x_next is not None:
                    zr_t, cp_t = zr_next, cp_next

    nc.compile()
    return nc


def _get_prog():
    global _PROG
    if _PROG is None:
        _PROG = _build()
    return _PROG


def _make_in_maps(video, Wz, bz, Wr, br, Wh, bh):
    w6 = np.concatenate(
        [
            Wz[:, :C].T, Wr[:, :C].T, Wh[:, :C].T,
            Wz[:, C:].T, Wr[:, C:].T, Wh[:, C:].T,
        ],
        axis=1,
    ).astype(np.float16)
    b3 = np.stack([bz, br, bh, -bz], axis=1).astype(np.float32)
    in_maps = []
    for core in range(NCORES):
        b_, q = divmod(core, 4)
        xs = np.ascontiguousarray(
            video[b_, :, :, q * HQ : (q + 1) * HQ, :]
        ).reshape(T, C, P).astype(np.float16)
        in_maps.append({"x_seq": xs, "wmats": w6, "biases": b3})
    return in_maps


def kernel(video, Wz, bz, Wr, br, Wh, bh):
    _ensure_paths()
    from concourse.bass_utils import run_bass_kernel_spmd

    video = np.asarray(video, dtype=np.float32)
    nc = _get_prog()
    in_maps = _make_in_maps(video, Wz, bz, Wr, br, Wh, bh)
    res = run_bass_kernel_spmd(nc, in_maps, list(range(NCORES)))

    out = np.empty((B, T, C, H, W), np.float32)
    for core in range(NCORES):
        b_, q = divmod(core, 4)
        out[b_, :, :, q * HQ : (q + 1) * HQ, :] = np.asarray(
            res.results[core]["out_seq"]
        ).astype(np.float32).reshape(T, C, HQ, W)
    return out


# revision 6
# speedup vs baseline: 1.6389x; 1.0024x over previous
"""ConvGRU Trainium2 kernel (v4).

video [B=2, T=16, C=128, H=64, W=64] f32; 1x1-conv GRU over T.
Sharding: data-parallel over (B x H/16) -> 8 cores, each core owns
P = 16*64 = 1024 pixels for all T; weights replicated.

Per core, per timestep (pixels on the free dim, channels on partitions):
    zr_pre = [Wzx@x + Wzh@h | Wrx@x + Wrh@h]      (PE, fp16 in / fp32 psum)
    z = sigmoid(zr_pre[:P] + bz); r = sigmoid(zr_pre[P:] + br)   (ACT)
    rh = r * h                                     (DVE)
    c = tanh(Whx@x + Whh@rh + bh)                  (PE + ACT)
    h = h + z * (c - h)                            (DVE, fp16 state)

The recurrence is latency-bound: each pixel group's step is a serial
cross-engine chain (h -> Wrh matmul -> sigmoid -> r*h -> Whh matmul ->
tanh -> blend -> h').  G=2 pixel groups form two independent chains
that interleave on the engines; all per-step ops stay PER-GROUP (a
merged-op variant that coupled the chains measured 33% slower).

Changes vs the 93.1us baseline (v1):
  - t=0 shortcut: h0 == 0, so closers, r-sigmoid and rh are skipped
    and h1 = sigmoid(pre_z + bz) * tanh(pre_c + bh)
  - output DMAs ride the otherwise-idle GpSimd queue so x prefetches
    never queue behind them on the sync HW queue
  - x prefetch issued two steps ahead (DMA landing latency is about
    one whole step period)
  - weight DMA split across the sync + gpsimd queues (x-side first)
    so x0 lands earlier; PE warmup matmuls run against a memset tile
    (no weight dependency) flipping the HAM clock gate during the DMAs
  - work pool bufs=3 so WAR buffer-recycle waits are stale and cheap

Numerics: fp16 matmul inputs/gates/state, fp32 PSUM accum + fp32 bias.
"""

import os
import sys

import numpy as np

B, T, C, H, W = 2, 16, 128, 64, 64
NCORES = 8
HQ = H // 4          # 16 rows of H per core (4 H-slices x 2 batches = 8 cores)
P = HQ * W           # 1024 pixels per core
G = 2                # pixel groups per step (independent recurrence chains)
PG = P // G          # 512 pixels per group

_PROG = None


def _ensure_paths():
    for p in ("/opt/trn_rl_repo",):
        if p not in sys.path and os.path.isdir(p):
            sys.path.append(p)


def _build():
    _ensure_paths()
    import concourse.bacc as bacc
    import concourse.tile as tile
    from concourse import mybir

    f32 = mybir.dt.float32
    f16 = mybir.dt.float16
    AF = mybir.ActivationFunctionType

    nc = bacc.Bacc(
        "TRN2", target_bir_lowering=False, debug=False, num_devices=NCORES
    )
    x_dram = nc.dram_tensor("x_seq", [T, C, P], f16, kind="ExternalInput")
    w_dram = nc.dram_tensor("wmats", [C, 6 * C], f16, kind="ExternalInput")
    b_dram = nc.dram_tensor("biases", [C, 4], f32, kind="ExternalInput")
    o_dram = nc.dram_tensor("out_seq", [T, C, P], f16, kind="ExternalOutput")

    x_ap = x_dram.ap()
    w_ap = w_dram.ap()
    b_ap = b_dram.ap()
    o_ap = o_dram.ap()

    # weight order in wmats: x-side first so its DMA can land first
    WZX, WRX, WHX, WZH, WRH, WHH = range(6)

    with tile.TileContext(nc) as tc:
        with (
            tc.tile_pool(name="consts", bufs=1) as consts,
            tc.tile_pool(name="xin", bufs=4) as xpool,
            tc.tile_pool(name="state", bufs=2) as spool,
            tc.tile_pool(name="work", bufs=3) as wk,
            tc.tile_pool(name="ps", bufs=1, space="PSUM") as ps,
        ):
            wt = consts.tile([C, 6 * C], f16)
            bt = consts.tile([C, 4], f32)
            nc.sync.dma_start(bt[:], b_ap[:])
            nc.sync.dma_start(wt[:, : 3 * C], w_ap[:, : 3 * C])
            nc.gpsimd.dma_start(wt[:, 3 * C :], w_ap[:, 3 * C :])

            def wslice(i):
                return wt[:, i * C : (i + 1) * C]

            def load_x(t):
                xt = xpool.tile([C, P], f16, tag="x")
                nc.sync.dma_start(xt[:], x_ap[t])
                return xt

            x_tiles = {0: load_x(0), 1: load_x(1), 2: load_x(2)}

            # -- warmup: ramp the PE clock gate with matmuls that only
            #    depend on a memset tile, while the input DMAs fly --
            warm16 = wk.tile([C, PG], f16, tag="warm")
            nc.vector.memset(warm16[:], 0.0)
            cwarm = [None, None]
            for g in range(G):
                cwarm[g] = ps.tile(
                    [C, PG], f32, tag=f"c_{g}", bufs=2, name=f"cwarm_{g}"
                )
            for i in range(6):
                nc.tensor.matmul(
                    cwarm[i % 2][:], warm16[:, :C], warm16[:],
                    start=True, stop=True,
                )
            # preload the ACT sigmoid/tanh table early
            wtmp = wk.tile([C, PG], f16, tag="scratch")
            nc.scalar.activation(
                wtmp[:], cwarm[0][:], AF.Sigmoid, bias=bt[:, 0:1]
            )

            # ---- t = 0: h0 == 0, so no closers / r-gate / rh ----
            x0 = x_tiles[0]
            zr0 = [None, None]
            for g in range(G):
                zrt = ps.tile([C, 2 * PG], f32, tag=f"zr_{g}", name=f"zr0_{g}")
                nc.tensor.matmul(
                    zrt[:, :PG], wslice(WZX), x0[:, g * PG : (g + 1) * PG],
                    start=True, stop=True,
                )
                zr0[g] = zrt
            c0 = [None, None]
            for g in range(G):
                cp = ps.tile([C, PG], f32, tag=f"c_{g}", bufs=2)
                nc.tensor.matmul(
                    cp[:], wslice(WHX), x0[:, g * PG : (g + 1) * PG],
                    start=True, stop=True,
                )
                c0[g] = cp
            h16 = [None, None]
            for g in range(G):
                zt = wk.tile([C, PG], f16, tag=f"zb_{g}")
                nc.scalar.activation(
                    zt[:], zr0[g][:, :PG], AF.Sigmoid, bias=bt[:, 0:1]
                )
                ct = wk.tile([C, PG], f16, tag=f"c16_{g}")
                nc.scalar.activation(ct[:], c0[g][:], AF.Tanh, bias=bt[:, 2:3])
                ht = spool.tile([C, PG], f16, tag=f"h16_{g}")
                nc.vector.tensor_mul(ht[:], zt[:], ct[:])
                h16[g] = ht
                nc.sync.dma_start(
                    o_ap[0, :, g * PG : (g + 1) * PG], ht[:]
                )

            def open_zr(xt, gorder):
                """Open z|r accumulations with the x-side contributions."""
                zr_t = [None] * G
                for g in gorder:
                    xs = xt[:, g * PG : (g + 1) * PG]
                    zrt = ps.tile([C, 2 * PG], f32, tag=f"zr_{g}", bufs=1,
                                  name=f"zr_t{g}")
                    nc.tensor.matmul(
                        zrt[:, PG:], wslice(WRX), xs, start=True, stop=False
                    )
                    nc.tensor.matmul(
                        zrt[:, :PG], wslice(WZX), xs, start=True, stop=False
                    )
                    zr_t[g] = zrt
                return zr_t

            def open_c(xt, gorder):
                cp_t = [None] * G
                for g in gorder:
                    xs = xt[:, g * PG : (g + 1) * PG]
                    cp = ps.tile([C, PG], f32, tag=f"c_{g}", bufs=2,
                                 name=f"c_t{g}")
                    nc.tensor.matmul(
                        cp[:], wslice(WHX), xs, start=True, stop=False
                    )
                    cp_t[g] = cp
                return cp_t

            first = list(range(G))
            x1 = x_tiles[1]
            zr_t = open_zr(x1, first)
            cp_t = open_c(x1, first)

            for t in range(1, T):
                go = first if t % 2 == 1 else first[::-1]
                x_next = x_tiles.get(t + 1)
                if t + 2 < T:
                    x_tiles[t + 2] = load_x(t + 2)

                # -- PE: close the r then z accumulations (chain head) --
                for g in go:
                    nc.tensor.matmul(
                        zr_t[g][:, PG:], wslice(WRH), h16[g][:],
                        start=False, stop=True,
                    )
                for g in go:
                    nc.tensor.matmul(
                        zr_t[g][:, :PG], wslice(WZH), h16[g][:],
                        start=False, stop=True,
                    )

                # -- ACT: r sigmoids first (they gate rh -> c matmul) --
                r16 = [None] * G
                for g in go:
                    rt = wk.tile([C, PG], f16, tag=f"r_{g}")
                    nc.scalar.activation(
                        rt[:], zr_t[g][:, PG:], AF.Sigmoid, bias=bt[:, 1:2]
                    )
                    r16[g] = rt

                rh16 = [None] * G
                for g in go:
                    rh = wk.tile([C, PG], f16, tag=f"rh_{g}")
                    nc.vector.tensor_mul(rh[:], r16[g][:], h16[g][:])
                    rh16[g] = rh

                for g in go:
                    nc.tensor.matmul(
                        cp_t[g][:], wslice(WHH), rh16[g][:],
                        start=False, stop=True,
                    )

                # next step's c openers can run any time (double-buffered)
                cp_next = open_c(x_next, go) if x_next is not None else None

                # -- ACT: zbar/tanh interleaved; zbar = 1-z = sigmoid(-pre)
                #    feeds the blend h' = zbar*h + (1-zbar)*c, whose only
                #    post-tanh serial ops are v = z*c and h' = u + v --
                zb16, c16 = [None] * G, [None] * G
                for g in go:
                    zbt = wk.tile([C, PG], f16, tag=f"zb_{g}")
                    nc.scalar.activation(
                        zbt[:], zr_t[g][:, :PG], AF.Sigmoid,
                        bias=bt[:, 3:4], scale=-1.0,
                    )
                    zb16[g] = zbt
                    ct = wk.tile([C, PG], f16, tag=f"c16_{g}")
                    nc.scalar.activation(
                        ct[:], cp_t[g][:], AF.Tanh, bias=bt[:, 2:3]
                    )
                    c16[g] = ct

                # next step's z|r openers (wait on this step's sigmoids)
                zr_next = open_zr(x_next, go) if x_next is not None else None

                # -- DVE mid-chain: u = zbar*h and z = 1-zbar overlap the
                #    tanh; only v and the final add trail it --
                u16, z16 = [None] * G, [None] * G
                for g in go:
                    ut = wk.tile([C, PG], f16, tag=f"u_{g}")
                    nc.vector.tensor_mul(ut[:], zb16[g][:], h16[g][:])
                    u16[g] = ut
                    zt = wk.tile([C, PG], f16, tag=f"z_{g}")
                    nc.vector.tensor_scalar(
                        zt[:], zb16[g][:], -1.0, 1.0,
                        mybir.AluOpType.mult, mybir.AluOpType.add,
                    )
                    z16[g] = zt

                for g in go:
                    v16 = wk.tile([C, PG], f16, tag=f"v_{g}")
                    nc.vector.tensor_mul(v16[:], z16[g][:], c16[g][:])
                    n16 = spool.tile([C, PG], f16, tag=f"h16_{g}")
                    nc.vector.tensor_add(n16[:], u16[g][:], v16[:])
                    h16[g] = n16
                    nc.sync.dma_start(
                        o_ap[t, :, g * PG : (g + 1) * PG], n16[:]
                    )

                x_tiles.pop(t - 1, None)
                if x_next is not None:
                    zr_t, cp_t = zr_next, cp_next

    nc.compile()
    return nc


def _get_prog():
    global _PROG
    if _PROG is None:
        _PROG = _build()
    return _PROG


def _make_in_maps(video, Wz, bz, Wr, br, Wh, bh):
    w6 = np.concatenate(
        [
            Wz[:, :C].T, Wr[:, :C].T, Wh[:, :C].T,
            Wz[:, C:].T, Wr[:, C:].T, Wh[:, C:].T,
        ],
        axis=1,
    ).astype(np.float16)
    b3 = np.stack([bz, br, bh, -bz], axis=1).astype(np.float32)
    in_maps = []
    for core in range(NCORES):
        b_, q = divmod(core, 4)
        xs = np.ascontiguousarray(
            video[b_, :, :, q * HQ : (q + 1) * HQ, :]
        ).reshape(T, C, P).astype(np.float16)
        in_maps.append({"x_seq": xs, "wmats": w6, "biases": b3})
    return in_maps


def kernel(video, Wz, bz, Wr, br, Wh, bh):
    _ensure_paths()
    from concourse.bass_utils import run_bass_kernel_spmd

    video = np.asarray(video, dtype=np.float32)
    nc = _get_prog()
    in_maps = _make_in_maps(video, Wz, bz, Wr, br, Wh, bh)
    res = run_bass_kernel_spmd(nc, in_maps, list(range(NCORES)))

    out = np.empty((B, T, C, H, W), np.float32)
    for core in range(NCORES):
        b_, q = divmod(core, 4)
        out[b_, :, :, q * HQ : (q + 1) * HQ, :] = np.asarray(
            res.results[core]["out_seq"]
        ).astype(np.float32).reshape(T, C, HQ, W)
    return out


# revision 7
# speedup vs baseline: 1.6779x; 1.0238x over previous
"""ConvGRU Trainium2 kernel (v4).

video [B=2, T=16, C=128, H=64, W=64] f32; 1x1-conv GRU over T.
Sharding: data-parallel over (B x H/16) -> 8 cores, each core owns
P = 16*64 = 1024 pixels for all T; weights replicated.

Per core, per timestep (pixels on the free dim, channels on partitions):
    zr_pre = [Wzx@x + Wzh@h | Wrx@x + Wrh@h]      (PE, fp16 in / fp32 psum)
    z = sigmoid(zr_pre[:P] + bz); r = sigmoid(zr_pre[P:] + br)   (ACT)
    rh = r * h                                     (DVE)
    c = tanh(Whx@x + Whh@rh + bh)                  (PE + ACT)
    h = h + z * (c - h)                            (DVE, fp16 state)

The recurrence is latency-bound: each pixel group's step is a serial
cross-engine chain (h -> Wrh matmul -> sigmoid -> r*h -> Whh matmul ->
tanh -> blend -> h').  G=2 pixel groups form two independent chains
that interleave on the engines; all per-step ops stay PER-GROUP (a
merged-op variant that coupled the chains measured 33% slower).

Changes vs the 93.1us baseline (v1):
  - t=0 shortcut: h0 == 0, so closers, r-sigmoid and rh are skipped
    and h1 = sigmoid(pre_z + bz) * tanh(pre_c + bh)
  - output DMAs ride the otherwise-idle GpSimd queue so x prefetches
    never queue behind them on the sync HW queue
  - x prefetch issued two steps ahead (DMA landing latency is about
    one whole step period)
  - weight DMA split across the sync + gpsimd queues (x-side first)
    so x0 lands earlier; PE warmup matmuls run against a memset tile
    (no weight dependency) flipping the HAM clock gate during the DMAs
  - work pool bufs=3 so WAR buffer-recycle waits are stale and cheap

Numerics: fp16 matmul inputs/gates/state, fp32 PSUM accum + fp32 bias.
"""

import os
import sys

import numpy as np

B, T, C, H, W = 2, 16, 128, 64, 64
NCORES = 8
HQ = H // 4          # 16 rows of H per core (4 H-slices x 2 batches = 8 cores)
P = HQ * W           # 1024 pixels per core
G = 2                # pixel groups per step (independent recurrence chains)
PG = P // G          # 512 pixels per group

_PROG = None


def _ensure_paths():
    for p in ("/opt/trn_rl_repo",):
        if p not in sys.path and os.path.isdir(p):
            sys.path.append(p)


def _build():
    _ensure_paths()
    import concourse.bacc as bacc
    import concourse.tile as tile
    from concourse import mybir

    f32 = mybir.dt.float32
    f16 = mybir.dt.float16
    AF = mybir.ActivationFunctionType

    nc = bacc.Bacc(
        "TRN2", target_bir_lowering=False, debug=False, num_devices=NCORES
    )
    x_dram = nc.dram_tensor("x_seq", [T, C, P], f16, kind="ExternalInput")
    w_dram = nc.dram_tensor("wmats", [C, 6 * C], f16, kind="ExternalInput")
    b_dram = nc.dram_tensor("biases", [C, 4], f32, kind="ExternalInput")
    o_dram = nc.dram_tensor("out_seq", [T, C, P], f16, kind="ExternalOutput")

    x_ap = x_dram.ap()
    w_ap = w_dram.ap()
    b_ap = b_dram.ap()
    o_ap = o_dram.ap()

    # weight order in wmats: x-side first so its DMA can land first
    WZX, WRX, WHX, WZH, WRH, WHH = range(6)

    with tile.TileContext(nc) as tc:
        with (
            tc.tile_pool(name="consts", bufs=1) as consts,
            tc.tile_pool(name="xin", bufs=4) as xpool,
            tc.tile_pool(name="state", bufs=4) as spool,
            tc.tile_pool(name="work", bufs=3) as wk,
            tc.tile_pool(name="ps", bufs=1, space="PSUM") as ps,
        ):
            wt = consts.tile([C, 6 * C], f16)
            bt = consts.tile([C, 4], f32)
            nc.sync.dma_start(bt[:], b_ap[:])
            nc.sync.dma_start(wt[:, : 3 * C], w_ap[:, : 3 * C])
            nc.gpsimd.dma_start(wt[:, 3 * C :], w_ap[:, 3 * C :])

            def wslice(i):
                return wt[:, i * C : (i + 1) * C]

            def load_x(t):
                xt = xpool.tile([C, P], f16, tag="x")
                nc.sync.dma_start(xt[:], x_ap[t])
                return xt

            x_tiles = {0: load_x(0), 1: load_x(1), 2: load_x(2)}

            # -- warmup: ramp the PE clock gate with matmuls that only
            #    depend on a memset tile, while the input DMAs fly --
            warm16 = wk.tile([C, PG], f16, tag="warm")
            nc.vector.memset(warm16[:], 0.0)
            cwarm = [None, None]
            for g in range(G):
                cwarm[g] = ps.tile(
                    [C, PG], f32, tag=f"c_{g}", bufs=2, name=f"cwarm_{g}"
                )
            for i in range(6):
                nc.tensor.matmul(
                    cwarm[i % 2][:], warm16[:, :C], warm16[:],
                    start=True, stop=True,
                )
            # preload the ACT sigmoid/tanh table early
            wtmp = wk.tile([C, PG], f16, tag="scratch")
            nc.scalar.activation(
                wtmp[:], cwarm[0][:], AF.Sigmoid, bias=bt[:, 0:1]
            )

            # ---- t = 0: h0 == 0, so no closers / r-gate / rh ----
            x0 = x_tiles[0]
            zr0 = [None, None]
            for g in range(G):
                zrt = ps.tile([C, 2 * PG], f32, tag=f"zr_{g}", name=f"zr0_{g}")
                nc.tensor.matmul(
                    zrt[:, :PG], wslice(WZX), x0[:, g * PG : (g + 1) * PG],
                    start=True, stop=True,
                )
                zr0[g] = zrt
            c0 = [None, None]
            for g in range(G):
                cp = ps.tile([C, PG], f32, tag=f"c_{g}", bufs=2)
                nc.tensor.matmul(
                    cp[:], wslice(WHX), x0[:, g * PG : (g + 1) * PG],
                    start=True, stop=True,
                )
                c0[g] = cp
            h16 = [None, None]
            for g in range(G):
                zt = wk.tile([C, PG], f16, tag=f"zb_{g}")
                nc.scalar.activation(
                    zt[:], zr0[g][:, :PG], AF.Sigmoid, bias=bt[:, 0:1]
                )
                ct = wk.tile([C, PG], f16, tag=f"c16_{g}")
                nc.scalar.activation(ct[:], c0[g][:], AF.Tanh, bias=bt[:, 2:3])
                ht = spool.tile([C, PG], f16, tag=f"h16_{g}")
                nc.vector.tensor_mul(ht[:], zt[:], ct[:])
                h16[g] = ht
                nc.sync.dma_start(
                    o_ap[0, :, g * PG : (g + 1) * PG], ht[:]
                )

            def open_zr(xt, gorder):
                """Open z|r accumulations with the x-side contributions."""
                zr_t = [None] * G
                for g in gorder:
                    xs = xt[:, g * PG : (g + 1) * PG]
                    zrt = ps.tile([C, 2 * PG], f32, tag=f"zr_{g}", bufs=1,
                                  name=f"zr_t{g}")
                    nc.tensor.matmul(
                        zrt[:, PG:], wslice(WRX), xs, start=True, stop=False
                    )
                    nc.tensor.matmul(
                        zrt[:, :PG], wslice(WZX), xs, start=True, stop=False
                    )
                    zr_t[g] = zrt
                return zr_t

            def open_c(xt, gorder):
                cp_t = [None] * G
                for g in gorder:
                    xs = xt[:, g * PG : (g + 1) * PG]
                    cp = ps.tile([C, PG], f32, tag=f"c_{g}", bufs=2,
                                 name=f"c_t{g}")
                    nc.tensor.matmul(
                        cp[:], wslice(WHX), xs, start=True, stop=False
                    )
                    cp_t[g] = cp
                return cp_t

            first = list(range(G))
            x1 = x_tiles[1]
            zr_t = open_zr(x1, first)
            cp_t = open_c(x1, first)

            for t in range(1, T):
                go = first if t % 2 == 1 else first[::-1]
                x_next = x_tiles.get(t + 1)
                if t + 2 < T:
                    x_tiles[t + 2] = load_x(t + 2)

                # -- PE: close the r then z accumulations (chain head) --
                for g in go:
                    nc.tensor.matmul(
                        zr_t[g][:, PG:], wslice(WRH), h16[g][:],
                        start=False, stop=True,
                    )
                for g in go:
                    nc.tensor.matmul(
                        zr_t[g][:, :PG], wslice(WZH), h16[g][:],
                        start=False, stop=True,
                    )

                # -- ACT: r sigmoids first (they gate rh -> c matmul) --
                r16 = [None] * G
                for g in go:
                    rt = wk.tile([C, PG], f16, tag=f"r_{g}")
                    nc.scalar.activation(
                        rt[:], zr_t[g][:, PG:], AF.Sigmoid, bias=bt[:, 1:2]
                    )
                    r16[g] = rt

                rh16 = [None] * G
                for g in go:
                    rh = wk.tile([C, PG], f16, tag=f"rh_{g}")
                    nc.vector.tensor_mul(rh[:], r16[g][:], h16[g][:])
                    rh16[g] = rh

                for g in go:
                    nc.tensor.matmul(
                        cp_t[g][:], wslice(WHH), rh16[g][:],
                        start=False, stop=True,
                    )

                # next step's c openers can run any time (double-buffered)
                cp_next = open_c(x_next, go) if x_next is not None else None

                # -- ACT: zbar/tanh interleaved; zbar = 1-z = sigmoid(-pre)
                #    feeds the blend h' = zbar*h + (1-zbar)*c, whose only
                #    post-tanh serial ops are v = z*c and h' = u + v --
                zb16, c16 = [None] * G, [None] * G
                for g in go:
                    zbt = wk.tile([C, PG], f16, tag=f"zb_{g}")
                    nc.scalar.activation(
                        zbt[:], zr_t[g][:, :PG], AF.Sigmoid,
                        bias=bt[:, 3:4], scale=-1.0,
                    )
                    zb16[g] = zbt
                    ct = wk.tile([C, PG], f16, tag=f"c16_{g}")
                    nc.scalar.activation(
                        ct[:], cp_t[g][:], AF.Tanh, bias=bt[:, 2:3]
                    )
                    c16[g] = ct

                # next step's z|r openers (wait on this step's sigmoids)
                zr_next = open_zr(x_next, go) if x_next is not None else None

                # -- DVE mid-chain: u = zbar*h and z = 1-zbar overlap the
                #    tanh; only v and the final add trail it --
                u16, z16 = [None] * G, [None] * G
                for g in go:
                    ut = wk.tile([C, PG], f16, tag=f"u_{g}")
                    nc.vector.tensor_mul(ut[:], zb16[g][:], h16[g][:])
                    u16[g] = ut
                    zt = wk.tile([C, PG], f16, tag=f"z_{g}")
                    nc.vector.tensor_scalar(
                        zt[:], zb16[g][:], -1.0, 1.0,
                        mybir.AluOpType.mult, mybir.AluOpType.add,
                    )
                    z16[g] = zt

                for g in go:
                    v16 = wk.tile([C, PG], f16, tag=f"v_{g}")
                    nc.vector.tensor_mul(v16[:], z16[g][:], c16[g][:])
                    n16 = spool.tile([C, PG], f16, tag=f"h16_{g}")
                    nc.vector.tensor_add(n16[:], u16[g][:], v16[:])
                    h16[g] = n16
                    nc.sync.dma_start(
                        o_ap[t, :, g * PG : (g + 1) * PG], n16[:]
                    )

                x_tiles.pop(t - 1, None)
                if x_next is not None:
                    zr_t, cp_t = zr_next, cp_next

    nc.compile()
    return nc


def _get_prog():
    global _PROG
    if _PROG is None:
        _PROG = _build()
    return _PROG


def _make_in_maps(video, Wz, bz, Wr, br, Wh, bh):
    w6 = np.concatenate(
        [
            Wz[:, :C].T, Wr[:, :C].T, Wh[:, :C].T,
            Wz[:, C:].T, Wr[:, C:].T, Wh[:, C:].T,
        ],
        axis=1,
    ).astype(np.float16)
    b3 = np.stack([bz, br, bh, -bz], axis=1).astype(np.float32)
    in_maps = []
    for core in range(NCORES):
        b_, q = divmod(core, 4)
        xs = np.ascontiguousarray(
            video[b_, :, :, q * HQ : (q + 1) * HQ, :]
        ).reshape(T, C, P).astype(np.float16)
        in_maps.append({"x_seq": xs, "wmats": w6, "biases": b3})
    return in_maps


def kernel(video, Wz, bz, Wr, br, Wh, bh):
    _ensure_paths()
    from concourse.bass_utils import run_bass_kernel_spmd

    video = np.asarray(video, dtype=np.float32)
    nc = _get_prog()
    in_maps = _make_in_maps(video, Wz, bz, Wr, br, Wh, bh)
    res = run_bass_kernel_spmd(nc, in_maps, list(range(NCORES)))

    out = np.empty((B, T, C, H, W), np.float32)
    for core in range(NCORES):
        b_, q = divmod(core, 4)
        out[b_, :, :, q * HQ : (q + 1) * HQ, :] = np.asarray(
            res.results[core]["out_seq"]
        ).astype(np.float32).reshape(T, C, HQ, W)
    return out
